# revision 8
# baseline (speedup 1.0000x reference)
"""Bernoulli edge-sampling kernel for Trainium2 (8 NeuronCores, SPMD row-sharded).

Reference computation (all f32):
    s      = sigmoid(x)
    logits = log(s/(1-s)) + log(u/(1-u))        # == x + logit(u) up to rounding
    s2     = sigmoid(logits / 0.5)              # == sigmoid(2x + 2c), c = logit(u)
    mask   = s2 > 0.5                           # == (2x + 2c) > 0 == x > -c
    w      = where(mask, s2, 0)

So the whole chain is one activation: w = sigmoid(2x + 2c) * 1[x > -c].
The ScalarE activation's free affine (func(in*scale + bias)) computes 2x+2c for
free; VectorE computes the indicator and the masked multiply.  The scalar c is
passed as a device input (not an immediate) so the NEFF is noise-independent.

mask is recovered on host as (w != 0): when x > -c the stored weight is
sigmoid(nonneg) >= ~0.5 > 0, and otherwise w is exactly 0.
"""

import sys

sys.path.insert(0, "/opt/trn_rl_repo")

import numpy as np

N = 8192
N_CORES = 8
ROWS = N // N_CORES  # 1024 rows per core
P = 128  # SBUF partitions
F = 4096  # free-dim tile size
TRACE = False  # test.py sets True to capture an NTFF profile
TMPDIR = None  # test.py may set a dir so trace artifacts persist
LAST_RESULTS = None  # BassKernelResults of the last kernel() call (for test.py)

_CACHE = {}


def _build_bass():
    """Build + compile the single-core Bass program (same NEFF on all 8 cores)."""
    import concourse.bacc as bacc
    import concourse.tile as tile
    from concourse import mybir

    nc = bacc.Bacc("TRN2", target_bir_lowering=False, debug=False)

    x = nc.dram_tensor("x", [ROWS, N], mybir.dt.float32, kind="ExternalInput")
    params = nc.dram_tensor("params", [P, 2], mybir.dt.float32, kind="ExternalInput")
    w = nc.dram_tensor("w", [ROWS, N], mybir.dt.float32, kind="ExternalOutput")

    xv = x.ap().rearrange("(t p) n -> t p n", p=P)  # [ROWS/P, P, N]
    wv = w.ap().rearrange("(t p) n -> t p n", p=P)

    # (row_tile, col_start, col_width) work list: full-width 8192 tiles for
    # the body (4MB contiguous DMAs, fewest instructions/semaphores), the
    # last row-tile split smaller so the final store chain drains quickly.
    work = [(t, 0, N) for t in range(ROWS // P - 1)]
    work += [(ROWS // P - 1, 0, N // 2), (ROWS // P - 1, N // 2, N // 4),
             (ROWS // P - 1, 3 * N // 4, N // 4)]

    with tile.TileContext(nc) as tc:
        with (
            tc.tile_pool(name="const", bufs=1) as cpool,
            tc.tile_pool(name="xp", bufs=3) as xpool,
            tc.tile_pool(name="sp", bufs=2) as spool,
        ):
            par = cpool.tile([P, 2], mybir.dt.float32)
            nc.sync.dma_start(par[:], params.ap())
            bias2c = par[:, 0:1]  # 2c, broadcast across partitions
            negc = par[:, 1:2]  # -c

            for it, (t, c0, cw) in enumerate(work):
                ld, stq = (nc.sync, nc.scalar) if it % 2 else (nc.scalar, nc.sync)
                cols = slice(c0, c0 + cw)
                xt = xpool.tile([P, N], mybir.dt.float32, tag="x")
                ld.dma_start(xt[:, :cw], xv[t, :, cols])
                st = spool.tile([P, N], mybir.dt.float32, tag="s")
                nc.scalar.activation(
                    st[:, :cw],
                    xt[:, :cw],
                    mybir.ActivationFunctionType.Sigmoid,
                    bias=bias2c,
                    scale=2.0,
                )
                # st = (xt > -c) * st  — one fused DVE op
                nc.vector.scalar_tensor_tensor(
                    st[:, :cw],
                    xt[:, :cw],
                    negc,
                    st[:, :cw],
                    op0=mybir.AluOpType.is_gt,
                    op1=mybir.AluOpType.mult,
                )
                stq.dma_start(wv[t, :, cols], st[:, :cw])

    nc.compile()
    return nc


def kernel(similarities, noise):
    global LAST_RESULTS
    from concourse import bass_utils

    if "nc" not in _CACHE:
        _CACHE["nc"] = _build_bass()
    nc = _CACHE["nc"]

    x = np.ascontiguousarray(np.asarray(similarities, dtype=np.float32))
    u = np.float32(np.asarray(noise).reshape(-1)[0])
    c = np.float32(np.log(u / (np.float32(1.0) - u)))
    params = np.empty((P, 2), dtype=np.float32)
    params[:, 0] = np.float32(2.0) * c
    params[:, 1] = -c

    in_maps = [
        {"x": x[k * ROWS : (k + 1) * ROWS], "params": params} for k in range(N_CORES)
    ]
    res = bass_utils.run_bass_kernel_spmd(
        nc, in_maps, core_ids=list(range(N_CORES)), trace=TRACE, tmpdir=TMPDIR
    )
    LAST_RESULTS = res

    weights = np.concatenate([r["w"] for r in res.results], axis=0)
    mask = weights != np.float32(0.0)
    return weights, mask


# revision 10
# speedup vs baseline: 1.0015x; 1.0015x over previous
"""Bernoulli edge-sampling kernel for Trainium2 (8 NeuronCores, SPMD row-sharded).

Reference computation (all f32):
    s      = sigmoid(x)
    logits = log(s/(1-s)) + log(u/(1-u))        # == x + logit(u) up to rounding
    s2     = sigmoid(logits / 0.5)              # == sigmoid(2x + 2c), c = logit(u)
    mask   = s2 > 0.5                           # == (2x + 2c) > 0 == x > -c
    w      = where(mask, s2, 0)

So the whole chain is one activation: w = sigmoid(2x + 2c) * 1[x > -c].
The ScalarE activation's free affine (func(in*scale + bias)) computes 2x+2c for
free; VectorE computes the indicator and the masked multiply.  The scalar c is
passed as a device input (not an immediate) so the NEFF is noise-independent.

mask is recovered on host as (w != 0): when x > -c the stored weight is
sigmoid(nonneg) >= ~0.5 > 0, and otherwise w is exactly 0.
"""

import sys

sys.path.insert(0, "/opt/trn_rl_repo")

import numpy as np

N = 8192
N_CORES = 8
ROWS = N // N_CORES  # 1024 rows per core
P = 128  # SBUF partitions
F = 4096  # free-dim tile size
TRACE = False  # test.py sets True to capture an NTFF profile
TMPDIR = None  # test.py may set a dir so trace artifacts persist
LAST_RESULTS = None  # BassKernelResults of the last kernel() call (for test.py)

_CACHE = {}


def _build_bass():
    """Build + compile the single-core Bass program (same NEFF on all 8 cores)."""
    import concourse.bacc as bacc
    import concourse.tile as tile
    from concourse import mybir

    nc = bacc.Bacc("TRN2", target_bir_lowering=False, debug=False)

    x = nc.dram_tensor("x", [ROWS, N], mybir.dt.float32, kind="ExternalInput")
    params = nc.dram_tensor("params", [P, 2], mybir.dt.float32, kind="ExternalInput")
    w = nc.dram_tensor("w", [ROWS, N], mybir.dt.float32, kind="ExternalOutput")

    xv = x.ap().rearrange("(t p) n -> t p n", p=P)  # [ROWS/P, P, N]
    wv = w.ap().rearrange("(t p) n -> t p n", p=P)

    # (row_tile, col_start, col_width) work list: 2MB [128, 4096] tiles,
    # with the final tile split in half so the last store chain drains fast.
    work = []
    for t in range(ROWS // P):
        for j in range(N // F):
            work.append((t, j * F, F))
    work[-1:] = [(work[-1][0], work[-1][1], F // 2),
                 (work[-1][0], work[-1][1] + F // 2, F // 2)]

    with tile.TileContext(nc) as tc:
        with (
            tc.tile_pool(name="const", bufs=1) as cpool,
            tc.tile_pool(name="xp", bufs=6) as xpool,
            tc.tile_pool(name="sp", bufs=5) as spool,
        ):
            par = cpool.tile([P, 2], mybir.dt.float32)
            nc.sync.dma_start(par[:], params.ap())
            bias2c = par[:, 0:1]  # 2c, broadcast across partitions
            negc = par[:, 1:2]  # -c

            for it, (t, c0, cw) in enumerate(work):
                ld, stq = (nc.sync, nc.scalar) if it % 2 else (nc.scalar, nc.sync)
                cols = slice(c0, c0 + cw)
                xt = xpool.tile([P, F], mybir.dt.float32, tag="x")
                ld.dma_start(xt[:, :cw], xv[t, :, cols])
                st = spool.tile([P, F], mybir.dt.float32, tag="s")
                nc.scalar.activation(
                    st[:, :cw],
                    xt[:, :cw],
                    mybir.ActivationFunctionType.Sigmoid,
                    bias=bias2c,
                    scale=2.0,
                )
                # st = (xt > -c) * st  — one fused DVE op
                nc.vector.scalar_tensor_tensor(
                    st[:, :cw],
                    xt[:, :cw],
                    negc,
                    st[:, :cw],
                    op0=mybir.AluOpType.is_gt,
                    op1=mybir.AluOpType.mult,
                )
                stq.dma_start(wv[t, :, cols], st[:, :cw])

    nc.compile()
    return nc


def kernel(similarities, noise):
    global LAST_RESULTS
    from concourse import bass_utils

    if "nc" not in _CACHE:
        _CACHE["nc"] = _build_bass()
    nc = _CACHE["nc"]

    x = np.ascontiguousarray(np.asarray(similarities, dtype=np.float32))
    u = np.float32(np.asarray(noise).reshape(-1)[0])
    c = np.float32(np.log(u / (np.float32(1.0) - u)))
    params = np.empty((P, 2), dtype=np.float32)
    params[:, 0] = np.float32(2.0) * c
    params[:, 1] = -c

    in_maps = [
        {"x": x[k * ROWS : (k + 1) * ROWS], "params": params} for k in range(N_CORES)
    ]
    res = bass_utils.run_bass_kernel_spmd(
        nc, in_maps, core_ids=list(range(N_CORES)), trace=TRACE, tmpdir=TMPDIR
    )
    LAST_RESULTS = res

    weights = np.concatenate([r["w"] for r in res.results], axis=0)
    mask = weights != np.float32(0.0)
    return weights, mask


# revision 13
# speedup vs baseline: 1.1181x; 1.1164x over previous
"""Bernoulli edge-sampling kernel for Trainium2 (8 NeuronCores, SPMD row-sharded).

Reference computation (all f32):
    s      = sigmoid(x)
    logits = log(s/(1-s)) + log(u/(1-u))        # == x + logit(u) up to rounding
    s2     = sigmoid(logits / 0.5)              # == sigmoid(2x + 2c), c = logit(u)
    mask   = s2 > 0.5                           # == (2x + 2c) > 0 == x > -c
    w      = where(mask, s2, 0)

So the whole chain is one activation: w = sigmoid(2x + 2c) * 1[x > -c].
The ScalarE activation's free affine (func(in*scale + bias)) computes 2x+2c for
free; VectorE computes the indicator and the masked multiply.  The scalar c is
passed as a device input (not an immediate) so the NEFF is noise-independent.

mask is recovered on host as (w != 0): when x > -c the stored weight is
sigmoid(nonneg) >= ~0.5 > 0, and otherwise w is exactly 0.
"""

import sys

sys.path.insert(0, "/opt/trn_rl_repo")

import numpy as np

N = 8192
N_CORES = 8
ROWS = N // N_CORES  # 1024 rows per core
P = 128  # SBUF partitions
F = 4096  # free-dim tile size
TRACE = False  # test.py sets True to capture an NTFF profile
TRACE_CORES = None  # e.g. list(range(8)) to profile every core
TMPDIR = None  # test.py may set a dir so trace artifacts persist
LAST_RESULTS = None  # BassKernelResults of the last kernel() call (for test.py)

_CACHE = {}


def _build_bass():
    """Build + compile the single-core Bass program (same NEFF on all 8 cores)."""
    import concourse.bacc as bacc
    import concourse.tile as tile
    from concourse import mybir

    nc = bacc.Bacc("TRN2", target_bir_lowering=False, debug=False)

    x = nc.dram_tensor("x", [ROWS, N], mybir.dt.float32, kind="ExternalInput")
    params = nc.dram_tensor("params", [P, 2], mybir.dt.float32, kind="ExternalInput")
    w = nc.dram_tensor("w", [ROWS, N], mybir.dt.float32, kind="ExternalOutput")

    xv = x.ap().rearrange("(t p) n -> t p n", p=P)  # [ROWS/P, P, N]
    wv = w.ap().rearrange("(t p) n -> t p n", p=P)

    # (row_tile, col_start, col_width) work list: 2MB [128, 4096] tiles,
    # with the final tile split in half so the last store chain drains fast.
    work = []
    for t in range(ROWS // P):
        for j in range(N // F):
            work.append((t, j * F, F))
    work[-1:] = [(work[-1][0], work[-1][1], F // 2),
                 (work[-1][0], work[-1][1] + F // 2, F // 2)]

    with tile.TileContext(nc) as tc:
        with (
            tc.tile_pool(name="const", bufs=1) as cpool,
            tc.tile_pool(name="xp", bufs=6) as xpool,
            tc.tile_pool(name="sp", bufs=5) as spool,
        ):
            par = cpool.tile([P, 2], mybir.dt.float32)
            # SWDGE path: keeps the 8B param load off the HWDGE rings so the
            # first big tile load heads its ring queue
            nc.gpsimd.dma_start(par[:], params.ap())
            bias2c = par[:, 0:1]  # 2c, broadcast across partitions
            negc = par[:, 1:2]  # -c

            for it, (t, c0, cw) in enumerate(work):
                ld, stq = (nc.sync, nc.scalar) if it % 2 else (nc.scalar, nc.sync)
                cols = slice(c0, c0 + cw)
                xt = xpool.tile([P, F], mybir.dt.float32, tag="x")
                ld.dma_start(xt[:, :cw], xv[t, :, cols])
                st = spool.tile([P, F], mybir.dt.float32, tag="s")
                nc.scalar.activation(
                    st[:, :cw],
                    xt[:, :cw],
                    mybir.ActivationFunctionType.Sigmoid,
                    bias=bias2c,
                    scale=2.0,
                )
                # st = (xt > -c) * st  — one fused DVE op
                nc.vector.scalar_tensor_tensor(
                    st[:, :cw],
                    xt[:, :cw],
                    negc,
                    st[:, :cw],
                    op0=mybir.AluOpType.is_gt,
                    op1=mybir.AluOpType.mult,
                )
                stq.dma_start(wv[t, :, cols], st[:, :cw])

    nc.compile()
    return nc


def kernel(similarities, noise):
    global LAST_RESULTS
    from concourse import bass_utils

    if "nc" not in _CACHE:
        _CACHE["nc"] = _build_bass()
    nc = _CACHE["nc"]

    x = np.ascontiguousarray(np.asarray(similarities, dtype=np.float32))
    u = np.float32(np.asarray(noise).reshape(-1)[0])
    c = np.float32(np.log(u / (np.float32(1.0) - u)))
    params = np.empty((P, 2), dtype=np.float32)
    params[:, 0] = np.float32(2.0) * c
    params[:, 1] = -c

    in_maps = [
        {"x": x[k * ROWS : (k + 1) * ROWS], "params": params} for k in range(N_CORES)
    ]
    res = bass_utils.run_bass_kernel_spmd(
        nc,
        in_maps,
        core_ids=list(range(N_CORES)),
        trace=TRACE,
        trace_cores=TRACE_CORES,
        tmpdir=TMPDIR,
    )
    LAST_RESULTS = res

    weights = np.concatenate([r["w"] for r in res.results], axis=0)
    mask = weights != np.float32(0.0)
    return weights, mask
